# revision 27
# baseline (speedup 1.0000x reference)
"""Trainium2 Bass kernel for nn_ATT2 (gnn_message_passing).

Contract: kernel(**inputs) takes FULL unsharded inputs, returns FULL output.
Batch dim (64) is sharded 8-ways across cores; all params replicated.

Math (per batch, E [n=100, d=64]):
  eln      = LayerNorm_raw(E)        (affine folded into host consts)
  u'[j,m]  = sum_k W1g[m,k] eln[j,k] + Ctot[m]
  v'[i,m]  = sum_k W2g[m,k] eln[i,k]
  t3c[i,j,m] = sum_k W3c[m,k] E[i,k] E[j,k]   (W3c row-centered: LN mean
                                               term of pair-product cancels)
  r[i,j]   = rsqrt(var(e_i*e_j)+eps) via Gram matrices of E, E^2
  pre      = r*t3c + u'_j + v'_i
  z[i,j]   = sum_m w_sum[m] tanh(pre)    (b_sum softmax-invariant, dropped)
  alphas   = softmax_j(z, diag masked)
  ctx      = leaky_relu(e_i * (alphas @ E) + e_i^2)
           = leaky_relu(e_i * ((alphas @ E) + e_i))

Layout: i in partitions (100), free = (j, m) blocked over j (JB=16).
PSUM accumulation per block into one region psA:
  MM1:  psA  = t3c                  (lhsT=Et, rhs=mg[k,(j,m)]=W3cT[k,m]*Et[k,j])
  DVE:  X    = psA * (r-1)          (fp16 to SBUF; broadcast AP on rG-1)
  MM2:  psA += u'_j + v'_i          (lhsT=[v'T;ones], rhs=[ind_m;u'flat])
  MM3:  psA += eye100 @ X           (adds (r-1)*t3c => psA = r*t3c + u + v)
  ACT:  att  = tanh(psA)  (fp16)
  DVE:  wm   = att * w_bcast (fp16 2x); z[:,blk] = reduce_m(wm)
Softmax over j is free-dim (no transpose); ctx via alphas^T transpose matmul.
"""

import sys

sys.path.insert(0, "/opt/trn_rl_repo")

import numpy as np

import concourse.bass as bass
import concourse.bacc as bacc
import concourse.mybir as mybir
from concourse import tile
from concourse.bass_utils import run_bass_kernel_spmd

import ml_dtypes

F32 = mybir.dt.float32
F16 = mybir.dt.bfloat16
NPH = ml_dtypes.bfloat16
AF = mybir.ActivationFunctionType
OP = mybir.AluOpType
AX = mybir.AxisListType

D = 64
N = 100
BS = 64
NCORES = 8
BPC = BS // NCORES
LN_EPS = 1e-5
NEG_SLOPE = 0.01
JB = 16
NBLK = (N + JB - 1) // JB  # 7 (6 full + 1 of 4)

# engine per mg-build block: 'g' = gpsimd, 'v' = vector
MG_ENG = "g" * NBLK
# engine per wm block: 'v' = vector, 'g' = gpsimd (balance the two)
WM_ENG = "ggggggg"

_CACHE = {}

I32 = mybir.dt.int32
RSQRT_MAGIC = 0x5F3759DF


def _rsqrt(nc, pool, out, x, P, Fd, tag):
    """out = 1/sqrt(x) on the vector engine only (no ScalarE table set).

    Quake seed y0 = bitcast(magic - (bits(x) >> 1)), then two Newton
    passes y <- y * (1.5 - 0.5 * x * y^2)  (~1e-6 rel err for x in
    [1e-3, 1e3], plenty under the 2e-2 gate).
    """
    sh = pool.tile([P, Fd], I32, tag=f"{tag}_sh")
    nc.vector.tensor_scalar(
        sh[:, :], x[:, :].bitcast(I32), 1, None, OP.arith_shift_right
    )
    neg = pool.tile([P, Fd], I32, tag=f"{tag}_ng")
    nc.vector.tensor_scalar(
        neg[:, :], sh[:, :], -1, RSQRT_MAGIC, OP.mult, OP.add
    )
    y = neg[:, :].bitcast(F32)
    cur = y
    for it in range(2):
        y2 = pool.tile([P, Fd], F32, tag=f"{tag}_y2{it}")
        nc.vector.tensor_tensor(y2[:, :], cur, cur, OP.mult)
        xy2 = pool.tile([P, Fd], F32, tag=f"{tag}_xy{it}")
        nc.vector.tensor_tensor(xy2[:, :], x[:, :], y2[:, :], OP.mult)
        half = pool.tile([P, Fd], F32, tag=f"{tag}_hf{it}")
        nc.vector.tensor_scalar(
            half[:, :], xy2[:, :], -0.5, 1.5, OP.mult, OP.add
        )
        nxt = out if it == 1 else pool.tile([P, Fd], F32, tag=f"{tag}_nx{it}")
        nc.vector.tensor_tensor(
            nxt[:, :] if it == 1 else nxt[:, :], cur, half[:, :], OP.mult
        )
        cur = nxt[:, :]


def build_program():
    nc = bacc.Bacc()

    emb = nc.declare_dram_parameter("emb", [BPC, N, D], F32, isOutput=False)
    embt = nc.declare_dram_parameter("embt", [BPC, D, N], F16, isOutput=False)
    w3t = nc.declare_dram_parameter("w3t", [D, D], F16, isOutput=False)
    w1cp = nc.declare_dram_parameter("w1cp", [D + 1, D], F16, isOutput=False)
    w2gt = nc.declare_dram_parameter("w2gt", [D, D], F16, isOutput=False)
    indm = nc.declare_dram_parameter("indm", [D, N * D], F16, isOutput=False)
    eye100 = nc.declare_dram_parameter("eye100", [N, N], F16, isOutput=False)
    wrep = nc.declare_dram_parameter("wrep", [N, D], F16, isOutput=False)
    maskd = nc.declare_dram_parameter("maskd", [N, N], F32, isOutput=False)
    out = nc.declare_dram_parameter("out", [BPC, N, D], F32, isOutput=True)

    with tile.TileContext(nc) as tc:
        with (
            tc.tile_pool(name="const", bufs=1) as constp,
            tc.tile_pool(name="mov", bufs=1) as movp,
            tc.tile_pool(name="batch", bufs=2) as bp,
            tc.tile_pool(name="blk", bufs=3) as blkp,
            tc.tile_pool(name="ps_big", bufs=2, space="PSUM") as psAp,
            tc.tile_pool(name="ps_gram", bufs=1, space="PSUM") as psGp,
            tc.tile_pool(name="ps_sm", bufs=1, space="PSUM") as psTp,
            tc.tile_pool(name="dram", bufs=2, space="DRAM") as dramp,
        ):
            # ---- one-time constants ----
            c_w3t = constp.tile([D, D], F16, tag="c_w3t")
            nc.sync.dma_start(out=c_w3t[:, :], in_=w3t[:, :])
            c_w1cp = constp.tile([D + 1, D], F16, tag="c_w1cp")
            nc.sync.dma_start(out=c_w1cp[:, :], in_=w1cp[:, :])
            c_w2gt = constp.tile([D, D], F16, tag="c_w2gt")
            nc.sync.dma_start(out=c_w2gt[:, :], in_=w2gt[:, :])
            c_eye = constp.tile([N, N], F16, tag="c_eye")
            nc.sync.dma_start(out=c_eye[:, :], in_=eye100[:, :])
            c_wrep = constp.tile([N, D], F16, tag="c_wrep")
            nc.sync.dma_start(out=c_wrep[:, :], in_=wrep[:, :])
            c_mask = constp.tile([N, N], F32, tag="c_mask")
            nc.sync.dma_start(out=c_mask[:, :], in_=maskd[:, :])
            c_eps1 = constp.tile([N, 1], F32, tag="c_eps1")
            nc.vector.memset(c_eps1[:, :], LN_EPS)
            # movA/movB: rows 0..63 = indm const, row 64 = u'flat per batch
            movs = []
            for mi in range(2):
                mv = movp.tile([D + 1, N * D], F16, tag=f"mov{mi}")
                nc.sync.dma_start(out=mv[:D, :], in_=indm[:, :])
                movs.append(mv)

            for b in range(BPC):
                mov = movs[b % 2]
                # ---- loads ----
                E = bp.tile([N, D], F32, tag="E")
                nc.sync.dma_start(out=E[:, :], in_=emb[b, :, :])
                Et = bp.tile([D, N], F16, tag="Et")
                nc.sync.dma_start(out=Et[:, :], in_=embt[b, :, :])

                # ---- row stats & eln ----
                E2s = bp.tile([N, D], F32, tag="E2s")
                nc.vector.tensor_tensor(E2s[:, :], E[:, :], E[:, :], OP.mult)
                qsum = bp.tile([N, 1], F32, tag="qsum")
                nc.vector.tensor_reduce(qsum[:, :], E2s[:, :], AX.X, OP.add)
                msum = bp.tile([N, 1], F32, tag="msum")
                nc.vector.tensor_reduce(msum[:, :], E[:, :], AX.X, OP.add)
                mean = bp.tile([N, 1], F32, tag="mean")
                nc.vector.tensor_scalar_mul(mean[:, :], msum[:, :], 1.0 / D)
                msq = bp.tile([N, 1], F32, tag="msq")
                nc.vector.tensor_tensor(msq[:, :], mean[:, :], mean[:, :], OP.mult)
                var = bp.tile([N, 1], F32, tag="var")
                nc.vector.scalar_tensor_tensor(
                    var[:, :], qsum[:, :], 1.0 / D, msq[:, :], OP.mult, OP.subtract
                )
                vare = bp.tile([N, 1], F32, tag="vare")
                nc.vector.tensor_scalar(
                    vare[:, :], var[:, :], float(LN_EPS), None, OP.add
                )
                rstd = bp.tile([N, 1], F32, tag="rstd")
                _rsqrt(nc, bp, rstd, vare, N, 1, "rsA")
                eln = bp.tile([N, D], F16, tag="eln")
                nc.vector.tensor_scalar(
                    eln[:, :], E[:, :], mean[:, :], rstd[:, :], OP.subtract, OP.mult
                )

                # ---- transpose eln (PE), build elnst [65,100] ----
                psT1 = psTp.tile([D, N], F16, tag="psT")
                nc.tensor.transpose(psT1[:, :], eln[:, :], c_eye[:, :])
                elnst = bp.tile([D + 1, N], F16, tag="elnst")
                nc.scalar.activation(elnst[:D, :], psT1[:, :], AF.Copy)
                nc.vector.memset(elnst[D : D + 1, :], 1.0)
                E2t = bp.tile([D, N], F16, tag="E2t")
                nc.vector.tensor_tensor(E2t[:, :], Et[:, :], Et[:, :], OP.mult)

                # ---- pair-stats r via Gram matmuls ----
                psG = psGp.tile([N, 2 * N], F32, tag="psG")
                nc.tensor.matmul(psG[:, :N], Et[:, :], Et[:, :], start=True, stop=True)
                nc.tensor.matmul(
                    psG[:, N : 2 * N], E2t[:, :], E2t[:, :], start=True, stop=True
                )
                muG = bp.tile([N, N], F32, tag="muG")
                nc.vector.tensor_scalar_mul(muG[:, :], psG[:, :N], 1.0 / D)
                musq = bp.tile([N, N], F32, tag="musq")
                nc.vector.tensor_tensor(musq[:, :], muG[:, :], muG[:, :], OP.mult)
                varG = bp.tile([N, N], F32, tag="varG")
                nc.vector.scalar_tensor_tensor(
                    varG[:, :], psG[:, N : 2 * N], 1.0 / D, musq[:, :],
                    OP.mult, OP.subtract,
                )
                varGe = bp.tile([N, N], F32, tag="varGe")
                nc.vector.tensor_scalar(
                    varGe[:, :], varG[:, :], float(LN_EPS), None, OP.add
                )
                rGf = bp.tile([N, N], F32, tag="rGf")
                _rsqrt(nc, bp, rGf, varGe, N, N, "rsB")
                # r-1: psA keeps t3c; adding X=(r-1)*t3c + uv yields r*t3c+uv
                rGm1 = bp.tile([N, N], F16, tag="rGm1")
                nc.vector.tensor_scalar(
                    rGm1[:, :], rGf[:, :], 1.0, None, OP.subtract
                )

                # ---- u' (to mov row 64) and v'^T (to vstk) ----
                psu = psTp.tile([N, D], F32, tag="psu")
                nc.tensor.matmul(
                    psu[:, :], elnst[:, :], c_w1cp[:, :], start=True, stop=True
                )
                us = bp.tile([N, D], F16, tag="us")
                nc.scalar.activation(us[:, :], psu[:, :], AF.Copy)
                vd = dramp.tile([1, N * D], F16, tag="vd")
                nc.sync.dma_start(
                    out=vd[:, :].rearrange("p (j m) -> (p j) m", m=D),
                    in_=us[:, :],
                )
                nc.sync.dma_start(out=mov[D : D + 1, :], in_=vd[:, :])
                psvT = psTp.tile([D, N], F32, tag="psvT")
                nc.tensor.matmul(
                    psvT[:, :], c_w2gt[:, :], elnst[:D, :], start=True, stop=True
                )
                vstk = bp.tile([D + 1, N], F16, tag="vstk")
                nc.scalar.activation(vstk[:D, :], psvT[:, :], AF.Copy)
                nc.vector.memset(vstk[D : D + 1, :], 1.0)

                Ef = bp.tile([N, D], F16, tag="Ef")
                nc.vector.tensor_scalar(Ef[:, :], E[:, :], 0.0, None, OP.add)

                # ---- block loop over j (software-pipelined: wm/reduce of
                # block t-1 are emitted during block t so Vector overlaps
                # Scalar's tanh and PE's next-block matmuls) ----
                z = bp.tile([N, N], F32, tag="z")
                atts = [None] * NBLK

                def emit_tail(t):
                    j0 = t * JB
                    jb = min(JB, N - j0)
                    F = jb * D
                    att = atts[t]
                    weng = nc.vector if WM_ENG[t] == "v" else nc.gpsimd
                    wm = blkp.tile([N, JB * D], F16, tag="wm")
                    weng.tensor_tensor(
                        wm[:, :F].rearrange("p (j m) -> p j m", m=D),
                        att[:, :F].rearrange("p (j m) -> p j m", m=D),
                        c_wrep[:, :].unsqueeze(1).broadcast_to([N, jb, D]),
                        OP.mult,
                    )
                    nc.vector.tensor_reduce(
                        z[:, j0 : j0 + jb],
                        wm[:, :F].rearrange("p (j m) -> p j m", m=D),
                        AX.X,
                        OP.add,
                    )

                def emit_head(t):
                    # mg build + MM1 for block t; returns psA tile
                    j0 = t * JB
                    jb = min(JB, N - j0)
                    F = jb * D
                    mg = blkp.tile([D, JB * D], F16, tag="mg")
                    eng = nc.gpsimd if MG_ENG[t] == "g" else nc.vector
                    eng.tensor_tensor(
                        mg[:, :F].rearrange("p (j m) -> p j m", m=D),
                        c_w3t[:, :].unsqueeze(1).broadcast_to([D, jb, D]),
                        Et[:, j0 : j0 + jb].unsqueeze(2).broadcast_to([D, jb, D]),
                        OP.mult,
                    )
                    psA = psAp.tile([N, JB * D], F32, tag="psA")
                    for h0 in range(0, F, 512):
                        h1 = min(h0 + 512, F)
                        nc.tensor.matmul(
                            psA[:, h0:h1], Et[:, :], mg[:, h0:h1],
                            start=True, stop=True,
                        )
                    return psA

                psAs = [None] * NBLK
                psAs[0] = emit_head(0)
                for t in range(NBLK):
                    j0 = t * JB
                    jb = min(JB, N - j0)
                    F = jb * D
                    psA = psAs[t]

                    X = blkp.tile([N, JB * D], F16, tag="X")
                    nc.vector.tensor_tensor(
                        X[:, :F].rearrange("p (j m) -> p j m", m=D),
                        psA[:, :F].rearrange("p (j m) -> p j m", m=D),
                        rGm1[:, j0 : j0 + jb].unsqueeze(2).broadcast_to([N, jb, D]),
                        OP.mult,
                    )
                    # next block's mg+MM1 go ahead of this block's uv/eyeX in
                    # the PE queue so the PE never stalls on X(t)
                    if t + 1 < NBLK:
                        psAs[t + 1] = emit_head(t + 1)
                    for h0 in range(0, F, 512):
                        h1 = min(h0 + 512, F)
                        nc.tensor.matmul(
                            psA[:, h0:h1], vstk[:, :],
                            mov[:, j0 * D + h0 : j0 * D + h1],
                            start=False, stop=False, skip_group_check=True,
                        )
                        nc.tensor.matmul(
                            psA[:, h0:h1], c_eye[:, :], X[:, h0:h1],
                            start=False, stop=True, skip_group_check=True,
                        )
                    att = blkp.tile([N, JB * D], F16, tag="att")
                    nc.scalar.activation(att[:, :F], psA[:, :F], AF.Tanh)
                    atts[t] = att
                    if t >= 1:
                        emit_tail(t - 1)
                emit_tail(NBLK - 1)

                # ---- softmax over j (free dim) ----
                zm = bp.tile([N, N], F32, tag="zm")
                nc.vector.tensor_tensor(zm[:, :], z[:, :], c_mask[:, :], OP.add)
                mx = bp.tile([N, 1], F32, tag="mx")
                nc.vector.tensor_reduce(mx[:, :], zm[:, :], AX.X, OP.max)
                mxn = bp.tile([N, 1], F32, tag="mxn")
                nc.vector.tensor_scalar_mul(mxn[:, :], mx[:, :], -1.0)
                p = bp.tile([N, N], F16, tag="p")
                nc.scalar.activation(p[:, :], zm[:, :], AF.Exp, bias=mxn[:, :])
                ssum = bp.tile([N, 1], F32, tag="ssum")
                nc.vector.tensor_reduce(ssum[:, :], p[:, :], AX.X, OP.add)
                sinv = bp.tile([N, 1], F32, tag="sinv")
                nc.vector.reciprocal(sinv[:, :], ssum[:, :])
                alph = bp.tile([N, N], F16, tag="alph")
                nc.vector.tensor_scalar_mul(alph[:, :], p[:, :], sinv[:, :])

                # ---- ctx = lrelu(E*(alphas@E + E)) ----
                psaT = psTp.tile([N, N], F16, tag="psT")
                nc.tensor.transpose(psaT[:, :], alph[:, :], c_eye[:, :])
                aT = bp.tile([N, N], F16, tag="aT")
                nc.scalar.activation(aT[:, :], psaT[:, :], AF.Copy)
                psc = psTp.tile([N, D], F32, tag="psu")
                nc.tensor.matmul(psc[:, :], aT[:, :], Ef[:, :], start=True, stop=True)
                s = bp.tile([N, D], F32, tag="s")
                nc.vector.tensor_tensor(s[:, :], psc[:, :], E[:, :], OP.add)
                ctxs = bp.tile([N, D], F32, tag="ctxs")
                nc.vector.tensor_tensor(ctxs[:, :], s[:, :], E[:, :], OP.mult)
                lo = bp.tile([N, D], F32, tag="lo")
                nc.vector.tensor_scalar_min(lo[:, :], ctxs[:, :], 0.0)
                hi = bp.tile([N, D], F32, tag="hi")
                nc.vector.tensor_scalar_max(hi[:, :], ctxs[:, :], 0.0)
                fin = bp.tile([N, D], F32, tag="fin")
                nc.vector.scalar_tensor_tensor(
                    fin[:, :], lo[:, :], NEG_SLOPE, hi[:, :], OP.mult, OP.add
                )
                nc.sync.dma_start(out=out[b, :, :], in_=fin[:, :])

    return nc


def _host_consts(ln_g, ln_b, w_ij, b_ij, w_sum, b_sum):
    d = D
    W1 = w_ij[:, :d]
    W2 = w_ij[:, d : 2 * d]
    W3 = w_ij[:, 2 * d :]
    W1g = W1 * ln_g[None, :]
    W2g = W2 * ln_g[None, :]
    W3g = W3 * ln_g[None, :]
    W3c = W3g - W3g.sum(axis=1)[:, None] / d
    Ctot = (W1 + W2 + W3) @ ln_b + b_ij
    w3t = np.ascontiguousarray(W3c.T)
    w1cp = np.vstack([W1g.T, Ctot[None, :]])
    w2gt = np.ascontiguousarray(W2g.T)
    indm = np.tile(np.eye(D, dtype=np.float16), (1, N))
    wrep = np.broadcast_to(w_sum[0][None, :], (N, D)).copy()
    maskd = (np.eye(N, dtype=np.float32) * -1e30).astype(np.float32)
    eye100 = np.eye(N, dtype=np.float16)
    return {
        "w3t": w3t.astype(np.float16),
        "w1cp": w1cp.astype(np.float16),
        "w2gt": w2gt.astype(np.float16),
        "indm": indm.astype(np.float16),
        "eye100": eye100,
        "wrep": wrep.astype(np.float16),
        "maskd": maskd,
    }


def _prepare(embeddings, ln_g, ln_b, w_ij, b_ij, w_sum, b_sum):
    embeddings = np.asarray(embeddings, dtype=np.float32)
    consts = _host_consts(
        np.asarray(ln_g, np.float32),
        np.asarray(ln_b, np.float32),
        np.asarray(w_ij, np.float32),
        np.asarray(b_ij, np.float32),
        np.asarray(w_sum, np.float32),
        np.asarray(b_sum, np.float32),
    )
    embt = np.ascontiguousarray(
        embeddings.transpose(0, 2, 1).astype(np.float16)
    )  # [BS, D, N]
    in_maps = []
    for c in range(NCORES):
        m = {
            "emb": np.ascontiguousarray(embeddings[c * BPC : (c + 1) * BPC]),
            "embt": np.ascontiguousarray(embt[c * BPC : (c + 1) * BPC]),
        }
        m.update(consts)
        in_maps.append(m)
    return in_maps


def kernel(embeddings, ln_g, ln_b, w_ij, b_ij, w_sum, b_sum, **kw):
    in_maps = _prepare(embeddings, ln_g, ln_b, w_ij, b_ij, w_sum, b_sum)
    if "nc" not in _CACHE:
        nc = build_program()
        if not nc.is_finalized():
            nc.finalize()
        _CACHE["nc"] = nc
    nc = _CACHE["nc"]
    res = run_bass_kernel_spmd(nc, in_maps, list(range(NCORES)))
    outs = [res.results[c]["out"] for c in range(NCORES)]
    return np.concatenate(outs, axis=0).astype(np.float32)


# revision 29
# speedup vs baseline: 1.0127x; 1.0127x over previous
"""Trainium2 Bass kernel for nn_ATT2 (gnn_message_passing).

Contract: kernel(**inputs) takes FULL unsharded inputs, returns FULL output.
Batch dim (64) is sharded 8-ways across cores; all params replicated.

Math (per batch, E [n=100, d=64]):
  eln      = LayerNorm_raw(E)        (affine folded into host consts)
  u'[j,m]  = sum_k W1g[m,k] eln[j,k] + Ctot[m]
  v'[i,m]  = sum_k W2g[m,k] eln[i,k]
  t3c[i,j,m] = sum_k W3c[m,k] E[i,k] E[j,k]   (W3c row-centered: LN mean
                                               term of pair-product cancels)
  r[i,j]   = rsqrt(var(e_i*e_j)+eps) via Gram matrices of E, E^2
  pre      = r*t3c + u'_j + v'_i
  z[i,j]   = sum_m w_sum[m] tanh(pre)    (b_sum softmax-invariant, dropped)
  alphas   = softmax_j(z, diag masked)
  ctx      = leaky_relu(e_i * (alphas @ E) + e_i^2)
           = leaky_relu(e_i * ((alphas @ E) + e_i))

Layout: i in partitions (100), free = (j, m) blocked over j (JB=16).
PSUM accumulation per block into one region psA:
  MM1:  psA  = t3c                  (lhsT=Et, rhs=mg[k,(j,m)]=W3cT[k,m]*Et[k,j])
  DVE:  X    = psA * (r-1)          (fp16 to SBUF; broadcast AP on rG-1)
  MM2:  psA += u'_j + v'_i          (lhsT=[v'T;ones], rhs=[ind_m;u'flat])
  MM3:  psA += eye100 @ X           (adds (r-1)*t3c => psA = r*t3c + u + v)
  ACT:  att  = tanh(psA)  (fp16)
  DVE:  wm   = att * w_bcast (fp16 2x); z[:,blk] = reduce_m(wm)
Softmax over j is free-dim (no transpose); ctx via alphas^T transpose matmul.
"""

import sys

sys.path.insert(0, "/opt/trn_rl_repo")

import numpy as np

import concourse.bass as bass
import concourse.bacc as bacc
import concourse.mybir as mybir
from concourse import tile
from concourse.bass_utils import run_bass_kernel_spmd

import ml_dtypes

F32 = mybir.dt.float32
F16 = mybir.dt.bfloat16
NPH = ml_dtypes.bfloat16
AF = mybir.ActivationFunctionType
OP = mybir.AluOpType
AX = mybir.AxisListType

D = 64
N = 100
BS = 64
NCORES = 8
BPC = BS // NCORES
LN_EPS = 1e-5
NEG_SLOPE = 0.01
JB = 16
NBLK = (N + JB - 1) // JB  # 7 (6 full + 1 of 4)

# engine per mg-build block: 'g' = gpsimd, 'v' = vector
MG_ENG = "g" * NBLK
# engine per wm block: 'v' = vector, 'g' = gpsimd (balance the two)
WM_ENG = "vgvgvgv"

_CACHE = {}

I32 = mybir.dt.int32
RSQRT_MAGIC = 0x5F3759DF


def _rsqrt(nc, pool, out, x, P, Fd, tag):
    """out = 1/sqrt(x) on the vector engine only (no ScalarE table set).

    Quake seed y0 = bitcast(magic - (bits(x) >> 1)), then two Newton
    passes y <- y * (1.5 - 0.5 * x * y^2)  (~1e-6 rel err for x in
    [1e-3, 1e3], plenty under the 2e-2 gate).
    """
    sh = pool.tile([P, Fd], I32, tag=f"{tag}_sh")
    nc.vector.tensor_scalar(
        sh[:, :], x[:, :].bitcast(I32), 1, None, OP.arith_shift_right
    )
    neg = pool.tile([P, Fd], I32, tag=f"{tag}_ng")
    nc.vector.tensor_scalar(
        neg[:, :], sh[:, :], -1, RSQRT_MAGIC, OP.mult, OP.add
    )
    y = neg[:, :].bitcast(F32)
    cur = y
    for it in range(2):
        y2 = pool.tile([P, Fd], F32, tag=f"{tag}_y2{it}")
        nc.vector.tensor_tensor(y2[:, :], cur, cur, OP.mult)
        xy2 = pool.tile([P, Fd], F32, tag=f"{tag}_xy{it}")
        nc.vector.tensor_tensor(xy2[:, :], x[:, :], y2[:, :], OP.mult)
        half = pool.tile([P, Fd], F32, tag=f"{tag}_hf{it}")
        nc.vector.tensor_scalar(
            half[:, :], xy2[:, :], -0.5, 1.5, OP.mult, OP.add
        )
        nxt = out if it == 1 else pool.tile([P, Fd], F32, tag=f"{tag}_nx{it}")
        nc.vector.tensor_tensor(
            nxt[:, :] if it == 1 else nxt[:, :], cur, half[:, :], OP.mult
        )
        cur = nxt[:, :]


def build_program():
    nc = bacc.Bacc()

    emb = nc.declare_dram_parameter("emb", [BPC, N, D], F32, isOutput=False)
    embt = nc.declare_dram_parameter("embt", [BPC, D, N], F16, isOutput=False)
    w3t = nc.declare_dram_parameter("w3t", [D, D], F16, isOutput=False)
    w1cp = nc.declare_dram_parameter("w1cp", [D + 1, D], F16, isOutput=False)
    w2gt = nc.declare_dram_parameter("w2gt", [D, D], F16, isOutput=False)
    indm = nc.declare_dram_parameter("indm", [D, N * D], F16, isOutput=False)
    eye100 = nc.declare_dram_parameter("eye100", [N, N], F16, isOutput=False)
    wrep = nc.declare_dram_parameter("wrep", [N, D], F16, isOutput=False)
    maskd = nc.declare_dram_parameter("maskd", [N, N], F32, isOutput=False)
    out = nc.declare_dram_parameter("out", [BPC, N, D], F32, isOutput=True)

    with tile.TileContext(nc) as tc:
        with (
            tc.tile_pool(name="const", bufs=1) as constp,
            tc.tile_pool(name="mov", bufs=1) as movp,
            tc.tile_pool(name="batch", bufs=3) as bp,
            tc.tile_pool(name="blk", bufs=4) as blkp,
            tc.tile_pool(name="ps_big", bufs=2, space="PSUM") as psAp,
            tc.tile_pool(name="ps_gram", bufs=1, space="PSUM") as psGp,
            tc.tile_pool(name="ps_sm", bufs=1, space="PSUM") as psTp,
            tc.tile_pool(name="dram", bufs=2, space="DRAM") as dramp,
        ):
            # ---- one-time constants ----
            c_w3t = constp.tile([D, D], F16, tag="c_w3t")
            nc.sync.dma_start(out=c_w3t[:, :], in_=w3t[:, :])
            c_w1cp = constp.tile([D + 1, D], F16, tag="c_w1cp")
            nc.sync.dma_start(out=c_w1cp[:, :], in_=w1cp[:, :])
            c_w2gt = constp.tile([D, D], F16, tag="c_w2gt")
            nc.sync.dma_start(out=c_w2gt[:, :], in_=w2gt[:, :])
            c_eye = constp.tile([N, N], F16, tag="c_eye")
            nc.sync.dma_start(out=c_eye[:, :], in_=eye100[:, :])
            c_wrep = constp.tile([N, D], F16, tag="c_wrep")
            nc.sync.dma_start(out=c_wrep[:, :], in_=wrep[:, :])
            c_mask = constp.tile([N, N], F32, tag="c_mask")
            nc.sync.dma_start(out=c_mask[:, :], in_=maskd[:, :])
            c_eps1 = constp.tile([N, 1], F32, tag="c_eps1")
            nc.vector.memset(c_eps1[:, :], LN_EPS)
            # movA/movB: rows 0..63 = indm const, row 64 = u'flat per batch
            movs = []
            for mi in range(3):
                mv = movp.tile([D + 1, N * D], F16, tag=f"mov{mi}")
                nc.sync.dma_start(out=mv[:D, :], in_=indm[:, :])
                movs.append(mv)

            for b in range(BPC):
                mov = movs[b % 3]
                # ---- loads ----
                E = bp.tile([N, D], F32, tag="E")
                nc.sync.dma_start(out=E[:, :], in_=emb[b, :, :])
                Et = bp.tile([D, N], F16, tag="Et")
                nc.sync.dma_start(out=Et[:, :], in_=embt[b, :, :])

                # ---- row stats & eln ----
                E2s = bp.tile([N, D], F32, tag="E2s")
                nc.vector.tensor_tensor(E2s[:, :], E[:, :], E[:, :], OP.mult)
                qsum = bp.tile([N, 1], F32, tag="qsum")
                nc.vector.tensor_reduce(qsum[:, :], E2s[:, :], AX.X, OP.add)
                msum = bp.tile([N, 1], F32, tag="msum")
                nc.vector.tensor_reduce(msum[:, :], E[:, :], AX.X, OP.add)
                mean = bp.tile([N, 1], F32, tag="mean")
                nc.vector.tensor_scalar_mul(mean[:, :], msum[:, :], 1.0 / D)
                msq = bp.tile([N, 1], F32, tag="msq")
                nc.vector.tensor_tensor(msq[:, :], mean[:, :], mean[:, :], OP.mult)
                var = bp.tile([N, 1], F32, tag="var")
                nc.vector.scalar_tensor_tensor(
                    var[:, :], qsum[:, :], 1.0 / D, msq[:, :], OP.mult, OP.subtract
                )
                vare = bp.tile([N, 1], F32, tag="vare")
                nc.vector.tensor_scalar(
                    vare[:, :], var[:, :], float(LN_EPS), None, OP.add
                )
                rstd = bp.tile([N, 1], F32, tag="rstd")
                _rsqrt(nc, bp, rstd, vare, N, 1, "rsA")
                eln = bp.tile([N, D], F16, tag="eln")
                nc.vector.tensor_scalar(
                    eln[:, :], E[:, :], mean[:, :], rstd[:, :], OP.subtract, OP.mult
                )

                # ---- transpose eln (PE), build elnst [65,100] ----
                psT1 = psTp.tile([D, N], F16, tag="psT")
                nc.tensor.transpose(psT1[:, :], eln[:, :], c_eye[:, :])
                elnst = bp.tile([D + 1, N], F16, tag="elnst")
                nc.scalar.activation(elnst[:D, :], psT1[:, :], AF.Copy)
                nc.vector.memset(elnst[D : D + 1, :], 1.0)
                E2t = bp.tile([D, N], F16, tag="E2t")
                nc.vector.tensor_tensor(E2t[:, :], Et[:, :], Et[:, :], OP.mult)

                # ---- pair-stats r via Gram matmuls ----
                psG = psGp.tile([N, 2 * N], F32, tag="psG")
                nc.tensor.matmul(psG[:, :N], Et[:, :], Et[:, :], start=True, stop=True)
                nc.tensor.matmul(
                    psG[:, N : 2 * N], E2t[:, :], E2t[:, :], start=True, stop=True
                )
                muG = bp.tile([N, N], F32, tag="muG")
                nc.vector.tensor_scalar_mul(muG[:, :], psG[:, :N], 1.0 / D)
                musq = bp.tile([N, N], F32, tag="musq")
                nc.vector.tensor_tensor(musq[:, :], muG[:, :], muG[:, :], OP.mult)
                varG = bp.tile([N, N], F32, tag="varG")
                nc.vector.scalar_tensor_tensor(
                    varG[:, :], psG[:, N : 2 * N], 1.0 / D, musq[:, :],
                    OP.mult, OP.subtract,
                )
                varGe = bp.tile([N, N], F32, tag="varGe")
                nc.vector.tensor_scalar(
                    varGe[:, :], varG[:, :], float(LN_EPS), None, OP.add
                )
                rGf = bp.tile([N, N], F32, tag="rGf")
                _rsqrt(nc, bp, rGf, varGe, N, N, "rsB")
                # r-1: psA keeps t3c; adding X=(r-1)*t3c + uv yields r*t3c+uv
                rGm1 = bp.tile([N, N], F16, tag="rGm1")
                nc.vector.tensor_scalar(
                    rGm1[:, :], rGf[:, :], 1.0, None, OP.subtract
                )

                # ---- u' (to mov row 64) and v'^T (to vstk) ----
                psu = psTp.tile([N, D], F32, tag="psu")
                nc.tensor.matmul(
                    psu[:, :], elnst[:, :], c_w1cp[:, :], start=True, stop=True
                )
                us = bp.tile([N, D], F16, tag="us")
                nc.scalar.activation(us[:, :], psu[:, :], AF.Copy)
                vd = dramp.tile([1, N * D], F16, tag="vd")
                nc.sync.dma_start(
                    out=vd[:, :].rearrange("p (j m) -> (p j) m", m=D),
                    in_=us[:, :],
                )
                nc.sync.dma_start(out=mov[D : D + 1, :], in_=vd[:, :])
                psvT = psTp.tile([D, N], F32, tag="psvT")
                nc.tensor.matmul(
                    psvT[:, :], c_w2gt[:, :], elnst[:D, :], start=True, stop=True
                )
                vstk = bp.tile([D + 1, N], F16, tag="vstk")
                nc.scalar.activation(vstk[:D, :], psvT[:, :], AF.Copy)
                nc.vector.memset(vstk[D : D + 1, :], 1.0)

                Ef = bp.tile([N, D], F16, tag="Ef")
                nc.vector.tensor_scalar(Ef[:, :], E[:, :], 0.0, None, OP.add)

                # ---- block loop over j (software-pipelined: wm/reduce of
                # block t-1 are emitted during block t so Vector overlaps
                # Scalar's tanh and PE's next-block matmuls) ----
                z = bp.tile([N, N], F32, tag="z")
                atts = [None] * NBLK

                def emit_tail(t):
                    j0 = t * JB
                    jb = min(JB, N - j0)
                    F = jb * D
                    att = atts[t]
                    weng = nc.vector if WM_ENG[t] == "v" else nc.gpsimd
                    wm = blkp.tile([N, JB * D], F16, tag="wm")
                    weng.tensor_tensor(
                        wm[:, :F].rearrange("p (j m) -> p j m", m=D),
                        att[:, :F].rearrange("p (j m) -> p j m", m=D),
                        c_wrep[:, :].unsqueeze(1).broadcast_to([N, jb, D]),
                        OP.mult,
                    )
                    nc.vector.tensor_reduce(
                        z[:, j0 : j0 + jb],
                        wm[:, :F].rearrange("p (j m) -> p j m", m=D),
                        AX.X,
                        OP.add,
                    )

                def emit_head(t):
                    # mg build + MM1 for block t; returns psA tile
                    j0 = t * JB
                    jb = min(JB, N - j0)
                    F = jb * D
                    mg = blkp.tile([D, JB * D], F16, tag="mg")
                    eng = nc.gpsimd if MG_ENG[t] == "g" else nc.vector
                    eng.tensor_tensor(
                        mg[:, :F].rearrange("p (j m) -> p j m", m=D),
                        c_w3t[:, :].unsqueeze(1).broadcast_to([D, jb, D]),
                        Et[:, j0 : j0 + jb].unsqueeze(2).broadcast_to([D, jb, D]),
                        OP.mult,
                    )
                    psA = psAp.tile([N, JB * D], F32, tag="psA")
                    for h0 in range(0, F, 512):
                        h1 = min(h0 + 512, F)
                        nc.tensor.matmul(
                            psA[:, h0:h1], Et[:, :], mg[:, h0:h1],
                            start=True, stop=True,
                        )
                    return psA

                psAs = [None] * NBLK
                psAs[0] = emit_head(0)
                for t in range(NBLK):
                    j0 = t * JB
                    jb = min(JB, N - j0)
                    F = jb * D
                    psA = psAs[t]

                    X = blkp.tile([N, JB * D], F16, tag="X")
                    nc.vector.tensor_tensor(
                        X[:, :F].rearrange("p (j m) -> p j m", m=D),
                        psA[:, :F].rearrange("p (j m) -> p j m", m=D),
                        rGm1[:, j0 : j0 + jb].unsqueeze(2).broadcast_to([N, jb, D]),
                        OP.mult,
                    )
                    # next block's mg+MM1 go ahead of this block's uv/eyeX in
                    # the PE queue so the PE never stalls on X(t)
                    if t + 1 < NBLK:
                        psAs[t + 1] = emit_head(t + 1)
                    for h0 in range(0, F, 512):
                        h1 = min(h0 + 512, F)
                        nc.tensor.matmul(
                            psA[:, h0:h1], vstk[:, :],
                            mov[:, j0 * D + h0 : j0 * D + h1],
                            start=False, stop=False, skip_group_check=True,
                        )
                        nc.tensor.matmul(
                            psA[:, h0:h1], c_eye[:, :], X[:, h0:h1],
                            start=False, stop=True, skip_group_check=True,
                        )
                    att = blkp.tile([N, JB * D], F16, tag="att")
                    nc.scalar.activation(att[:, :F], psA[:, :F], AF.Tanh)
                    atts[t] = att
                    if t >= 1:
                        emit_tail(t - 1)
                emit_tail(NBLK - 1)

                # ---- softmax over j (free dim) ----
                zm = bp.tile([N, N], F32, tag="zm")
                nc.vector.tensor_tensor(zm[:, :], z[:, :], c_mask[:, :], OP.add)
                mx = bp.tile([N, 1], F32, tag="mx")
                nc.vector.tensor_reduce(mx[:, :], zm[:, :], AX.X, OP.max)
                mxn = bp.tile([N, 1], F32, tag="mxn")
                nc.vector.tensor_scalar_mul(mxn[:, :], mx[:, :], -1.0)
                p = bp.tile([N, N], F16, tag="p")
                nc.scalar.activation(p[:, :], zm[:, :], AF.Exp, bias=mxn[:, :])
                ssum = bp.tile([N, 1], F32, tag="ssum")
                nc.vector.tensor_reduce(ssum[:, :], p[:, :], AX.X, OP.add)
                sinv = bp.tile([N, 1], F32, tag="sinv")
                nc.vector.reciprocal(sinv[:, :], ssum[:, :])
                alph = bp.tile([N, N], F16, tag="alph")
                nc.vector.tensor_scalar_mul(alph[:, :], p[:, :], sinv[:, :])

                # ---- ctx = lrelu(E*(alphas@E + E)) ----
                psaT = psTp.tile([N, N], F16, tag="psT")
                nc.tensor.transpose(psaT[:, :], alph[:, :], c_eye[:, :])
                aT = bp.tile([N, N], F16, tag="aT")
                nc.scalar.activation(aT[:, :], psaT[:, :], AF.Copy)
                psc = psTp.tile([N, D], F32, tag="psu")
                nc.tensor.matmul(psc[:, :], aT[:, :], Ef[:, :], start=True, stop=True)
                s = bp.tile([N, D], F32, tag="s")
                nc.vector.tensor_tensor(s[:, :], psc[:, :], E[:, :], OP.add)
                ctxs = bp.tile([N, D], F32, tag="ctxs")
                nc.vector.tensor_tensor(ctxs[:, :], s[:, :], E[:, :], OP.mult)
                lo = bp.tile([N, D], F32, tag="lo")
                nc.vector.tensor_scalar_min(lo[:, :], ctxs[:, :], 0.0)
                hi = bp.tile([N, D], F32, tag="hi")
                nc.vector.tensor_scalar_max(hi[:, :], ctxs[:, :], 0.0)
                fin = bp.tile([N, D], F32, tag="fin")
                nc.vector.scalar_tensor_tensor(
                    fin[:, :], lo[:, :], NEG_SLOPE, hi[:, :], OP.mult, OP.add
                )
                nc.sync.dma_start(out=out[b, :, :], in_=fin[:, :])

    return nc


def _host_consts(ln_g, ln_b, w_ij, b_ij, w_sum, b_sum):
    d = D
    W1 = w_ij[:, :d]
    W2 = w_ij[:, d : 2 * d]
    W3 = w_ij[:, 2 * d :]
    W1g = W1 * ln_g[None, :]
    W2g = W2 * ln_g[None, :]
    W3g = W3 * ln_g[None, :]
    W3c = W3g - W3g.sum(axis=1)[:, None] / d
    Ctot = (W1 + W2 + W3) @ ln_b + b_ij
    w3t = np.ascontiguousarray(W3c.T)
    w1cp = np.vstack([W1g.T, Ctot[None, :]])
    w2gt = np.ascontiguousarray(W2g.T)
    indm = np.tile(np.eye(D, dtype=np.float16), (1, N))
    wrep = np.broadcast_to(w_sum[0][None, :], (N, D)).copy()
    maskd = (np.eye(N, dtype=np.float32) * -1e30).astype(np.float32)
    eye100 = np.eye(N, dtype=np.float16)
    return {
        "w3t": w3t.astype(np.float16),
        "w1cp": w1cp.astype(np.float16),
        "w2gt": w2gt.astype(np.float16),
        "indm": indm.astype(np.float16),
        "eye100": eye100,
        "wrep": wrep.astype(np.float16),
        "maskd": maskd,
    }


def _prepare(embeddings, ln_g, ln_b, w_ij, b_ij, w_sum, b_sum):
    embeddings = np.asarray(embeddings, dtype=np.float32)
    consts = _host_consts(
        np.asarray(ln_g, np.float32),
        np.asarray(ln_b, np.float32),
        np.asarray(w_ij, np.float32),
        np.asarray(b_ij, np.float32),
        np.asarray(w_sum, np.float32),
        np.asarray(b_sum, np.float32),
    )
    embt = np.ascontiguousarray(
        embeddings.transpose(0, 2, 1).astype(np.float16)
    )  # [BS, D, N]
    in_maps = []
    for c in range(NCORES):
        m = {
            "emb": np.ascontiguousarray(embeddings[c * BPC : (c + 1) * BPC]),
            "embt": np.ascontiguousarray(embt[c * BPC : (c + 1) * BPC]),
        }
        m.update(consts)
        in_maps.append(m)
    return in_maps


def kernel(embeddings, ln_g, ln_b, w_ij, b_ij, w_sum, b_sum, **kw):
    in_maps = _prepare(embeddings, ln_g, ln_b, w_ij, b_ij, w_sum, b_sum)
    if "nc" not in _CACHE:
        nc = build_program()
        if not nc.is_finalized():
            nc.finalize()
        _CACHE["nc"] = nc
    nc = _CACHE["nc"]
    res = run_bass_kernel_spmd(nc, in_maps, list(range(NCORES)))
    outs = [res.results[c]["out"] for c in range(NCORES)]
    return np.concatenate(outs, axis=0).astype(np.float32)


# revision 34
# speedup vs baseline: 1.0180x; 1.0052x over previous
"""Trainium2 Bass kernel for nn_ATT2 (gnn_message_passing).

Contract: kernel(**inputs) takes FULL unsharded inputs, returns FULL output.
Batch dim (64) is sharded 8-ways across cores; all params replicated.

Math (per batch, E [n=100, d=64]):
  eln      = LayerNorm_raw(E)        (affine folded into host consts)
  u'[j,m]  = sum_k W1g[m,k] eln[j,k] + Ctot[m]
  v'[i,m]  = sum_k W2g[m,k] eln[i,k]
  t3c[i,j,m] = sum_k W3c[m,k] E[i,k] E[j,k]   (W3c row-centered: LN mean
                                               term of pair-product cancels)
  r[i,j]   = rsqrt(var(e_i*e_j)+eps) via Gram matrices of E, E^2
  pre      = r*t3c + u'_j + v'_i
  z[i,j]   = sum_m w_sum[m] tanh(pre)    (b_sum softmax-invariant, dropped)
  alphas   = softmax_j(z, diag masked)
  ctx      = leaky_relu(e_i * (alphas @ E) + e_i^2)
           = leaky_relu(e_i * ((alphas @ E) + e_i))

Layout: i in partitions (100), free = (j, m) blocked over j (JB=16).
PSUM accumulation per block into one region psA:
  MM1:  psA  = t3c                  (lhsT=Et, rhs=mg[k,(j,m)]=W3cT[k,m]*Et[k,j])
  DVE:  X    = psA * (r-1)          (fp16 to SBUF; broadcast AP on rG-1)
  MM2:  psA += u'_j + v'_i          (lhsT=[v'T;ones], rhs=[ind_m;u'flat])
  MM3:  psA += eye100 @ X           (adds (r-1)*t3c => psA = r*t3c + u + v)
  ACT:  att  = tanh(psA)  (fp16)
  DVE:  wm   = att * w_bcast (fp16 2x); z[:,blk] = reduce_m(wm)
Softmax over j is free-dim (no transpose); ctx via alphas^T transpose matmul.
"""

import sys

sys.path.insert(0, "/opt/trn_rl_repo")

import numpy as np

import concourse.bass as bass
import concourse.bacc as bacc
import concourse.mybir as mybir
from concourse import tile
from concourse.bass_utils import run_bass_kernel_spmd

import ml_dtypes

F32 = mybir.dt.float32
F16 = mybir.dt.bfloat16
NPH = ml_dtypes.bfloat16
AF = mybir.ActivationFunctionType
OP = mybir.AluOpType
AX = mybir.AxisListType

D = 64
N = 100
BS = 64
NCORES = 8
BPC = BS // NCORES
LN_EPS = 1e-5
NEG_SLOPE = 0.01
JB = 16
NBLK = (N + JB - 1) // JB  # 7 (6 full + 1 of 4)

# engine per mg-build block: 'g' = gpsimd, 'v' = vector
MG_ENG = "g" * NBLK
# engine per wm block: 'v' = vector, 'g' = gpsimd (balance the two)
WM_ENG = "vgvgvgv"

_CACHE = {}

I32 = mybir.dt.int32
RSQRT_MAGIC = 0x5F3759DF


def _rsqrt(nc, pool, out, x, P, Fd, tag):
    """out = 1/sqrt(x) on the vector engine only (no ScalarE table set).

    Quake seed y0 = bitcast(magic - (bits(x) >> 1)), then two Newton
    passes y <- y * (1.5 - 0.5 * x * y^2)  (~1e-6 rel err for x in
    [1e-3, 1e3], plenty under the 2e-2 gate).
    """
    sh = pool.tile([P, Fd], I32, tag=f"{tag}_sh")
    nc.vector.tensor_scalar(
        sh[:, :], x[:, :].bitcast(I32), 1, None, OP.arith_shift_right
    )
    neg = pool.tile([P, Fd], I32, tag=f"{tag}_ng")
    nc.vector.tensor_scalar(
        neg[:, :], sh[:, :], -1, RSQRT_MAGIC, OP.mult, OP.add
    )
    y = neg[:, :].bitcast(F32)
    cur = y
    for it in range(2):
        y2 = pool.tile([P, Fd], F32, tag=f"{tag}_y2{it}")
        nc.vector.tensor_tensor(y2[:, :], cur, cur, OP.mult)
        xy2 = pool.tile([P, Fd], F32, tag=f"{tag}_xy{it}")
        nc.vector.tensor_tensor(xy2[:, :], x[:, :], y2[:, :], OP.mult)
        half = pool.tile([P, Fd], F32, tag=f"{tag}_hf{it}")
        nc.vector.tensor_scalar(
            half[:, :], xy2[:, :], -0.5, 1.5, OP.mult, OP.add
        )
        nxt = out if it == 1 else pool.tile([P, Fd], F32, tag=f"{tag}_nx{it}")
        nc.vector.tensor_tensor(
            nxt[:, :] if it == 1 else nxt[:, :], cur, half[:, :], OP.mult
        )
        cur = nxt[:, :]


def build_program():
    nc = bacc.Bacc()

    emb = nc.declare_dram_parameter("emb", [BPC, N, D], F32, isOutput=False)
    embt = nc.declare_dram_parameter("embt", [BPC, D, N], F16, isOutput=False)
    w3t = nc.declare_dram_parameter("w3t", [D, D], F16, isOutput=False)
    w1cp = nc.declare_dram_parameter("w1cp", [D + 1, D], F16, isOutput=False)
    w2gt = nc.declare_dram_parameter("w2gt", [D, D], F16, isOutput=False)
    indm = nc.declare_dram_parameter("indm", [D, N * D], F16, isOutput=False)
    eye100 = nc.declare_dram_parameter("eye100", [N, N], F16, isOutput=False)
    wrep = nc.declare_dram_parameter("wrep", [N, D], F16, isOutput=False)
    maskd = nc.declare_dram_parameter("maskd", [N, N], F32, isOutput=False)
    out = nc.declare_dram_parameter("out", [BPC, N, D], F32, isOutput=True)

    with tile.TileContext(nc) as tc:
        with (
            tc.tile_pool(name="const", bufs=1) as constp,
            tc.tile_pool(name="mov", bufs=1) as movp,
            tc.tile_pool(name="batch", bufs=2) as bp,
            tc.tile_pool(name="blk", bufs=3) as blkp,
            tc.tile_pool(name="ps_big", bufs=2, space="PSUM") as psAp,
            tc.tile_pool(name="ps_gram", bufs=1, space="PSUM") as psGp,
            tc.tile_pool(name="ps_sm", bufs=1, space="PSUM") as psTp,
            tc.tile_pool(name="dram", bufs=2, space="DRAM") as dramp,
        ):
            # ---- one-time constants ----
            c_w3t = constp.tile([D, D], F16, tag="c_w3t")
            nc.sync.dma_start(out=c_w3t[:, :], in_=w3t[:, :])
            c_w1cp = constp.tile([D + 1, D], F16, tag="c_w1cp")
            nc.sync.dma_start(out=c_w1cp[:, :], in_=w1cp[:, :])
            c_w2gt = constp.tile([D, D], F16, tag="c_w2gt")
            nc.sync.dma_start(out=c_w2gt[:, :], in_=w2gt[:, :])
            c_eye = constp.tile([N, N], F16, tag="c_eye")
            nc.sync.dma_start(out=c_eye[:, :], in_=eye100[:, :])
            c_wrep = constp.tile([N, D], F16, tag="c_wrep")
            nc.sync.dma_start(out=c_wrep[:, :], in_=wrep[:, :])
            c_mask = constp.tile([N, N], F32, tag="c_mask")
            nc.sync.dma_start(out=c_mask[:, :], in_=maskd[:, :])
            c_eps1 = constp.tile([N, 1], F32, tag="c_eps1")
            nc.vector.memset(c_eps1[:, :], LN_EPS)
            # movA/movB: rows 0..63 = indm const, row 64 = u'flat per batch
            movs = []
            for mi in range(2):
                mv = movp.tile([D + 1, N * D], F16, tag=f"mov{mi}")
                nc.sync.dma_start(out=mv[:D, :], in_=indm[:, :])
                movs.append(mv)

            for b in range(BPC):
                mov = movs[b % 2]
                # ---- loads ----
                E = bp.tile([N, D], F32, tag="E")
                nc.sync.dma_start(out=E[:, :], in_=emb[b, :, :])
                Et = bp.tile([D, N], F16, tag="Et")
                nc.sync.dma_start(out=Et[:, :], in_=embt[b, :, :])

                # ---- row stats & eln ----
                E2s = bp.tile([N, D], F32, tag="E2s")
                nc.vector.tensor_tensor(E2s[:, :], E[:, :], E[:, :], OP.mult)
                qsum = bp.tile([N, 1], F32, tag="qsum")
                nc.vector.tensor_reduce(qsum[:, :], E2s[:, :], AX.X, OP.add)
                msum = bp.tile([N, 1], F32, tag="msum")
                nc.vector.tensor_reduce(msum[:, :], E[:, :], AX.X, OP.add)
                mean = bp.tile([N, 1], F32, tag="mean")
                nc.vector.tensor_scalar_mul(mean[:, :], msum[:, :], 1.0 / D)
                msq = bp.tile([N, 1], F32, tag="msq")
                nc.vector.tensor_tensor(msq[:, :], mean[:, :], mean[:, :], OP.mult)
                var = bp.tile([N, 1], F32, tag="var")
                nc.vector.scalar_tensor_tensor(
                    var[:, :], qsum[:, :], 1.0 / D, msq[:, :], OP.mult, OP.subtract
                )
                vare = bp.tile([N, 1], F32, tag="vare")
                nc.vector.tensor_scalar(
                    vare[:, :], var[:, :], float(LN_EPS), None, OP.add
                )
                rstd = bp.tile([N, 1], F32, tag="rstd")
                _rsqrt(nc, bp, rstd, vare, N, 1, "rsA")
                eln = bp.tile([N, D], F16, tag="eln")
                nc.vector.tensor_scalar(
                    eln[:, :], E[:, :], mean[:, :], rstd[:, :], OP.subtract, OP.mult
                )

                # ---- transpose eln (PE), build elnst [65,100] ----
                psT1 = psTp.tile([D, N], F16, tag="psT")
                nc.tensor.transpose(psT1[:, :], eln[:, :], c_eye[:, :])
                elnst = bp.tile([D + 1, N], F16, tag="elnst")
                nc.scalar.activation(elnst[:D, :], psT1[:, :], AF.Copy)
                nc.vector.memset(elnst[D : D + 1, :], 1.0)
                E2t = bp.tile([D, N], F16, tag="E2t")
                nc.vector.tensor_tensor(E2t[:, :], Et[:, :], Et[:, :], OP.mult)

                # ---- pair-stats r via Gram matmuls ----
                psG = psGp.tile([N, 2 * N], F32, tag="psG")
                nc.tensor.matmul(psG[:, :N], Et[:, :], Et[:, :], start=True, stop=True)
                nc.tensor.matmul(
                    psG[:, N : 2 * N], E2t[:, :], E2t[:, :], start=True, stop=True
                )
                muG = bp.tile([N, N], F32, tag="muG")
                nc.vector.tensor_scalar_mul(muG[:, :], psG[:, :N], 1.0 / D)
                musq = bp.tile([N, N], F32, tag="musq")
                nc.vector.tensor_tensor(musq[:, :], muG[:, :], muG[:, :], OP.mult)
                varG = bp.tile([N, N], F32, tag="varG")
                nc.vector.scalar_tensor_tensor(
                    varG[:, :], psG[:, N : 2 * N], 1.0 / D, musq[:, :],
                    OP.mult, OP.subtract,
                )
                varGe = bp.tile([N, N], F32, tag="varGe")
                nc.vector.tensor_scalar(
                    varGe[:, :], varG[:, :], float(LN_EPS), None, OP.add
                )
                rGf = bp.tile([N, N], F32, tag="rGf")
                _rsqrt(nc, bp, rGf, varGe, N, N, "rsB")
                # r-1: psA keeps t3c; adding X=(r-1)*t3c + uv yields r*t3c+uv
                rGm1 = bp.tile([N, N], F16, tag="rGm1")
                nc.vector.tensor_scalar(
                    rGm1[:, :], rGf[:, :], 1.0, None, OP.subtract
                )

                # ---- u' (to mov row 64) and v'^T (to vstk) ----
                psu = psTp.tile([N, D], F32, tag="psu")
                nc.tensor.matmul(
                    psu[:, :], elnst[:, :], c_w1cp[:, :], start=True, stop=True
                )
                us = bp.tile([N, D], F16, tag="us")
                nc.scalar.activation(us[:, :], psu[:, :], AF.Copy)
                vd = dramp.tile([1, N * D], F16, tag="vd")
                nc.sync.dma_start(
                    out=vd[:, :].rearrange("p (j m) -> (p j) m", m=D),
                    in_=us[:, :],
                )
                nc.sync.dma_start(out=mov[D : D + 1, :], in_=vd[:, :])
                psvT = psTp.tile([D, N], F32, tag="psvT")
                nc.tensor.matmul(
                    psvT[:, :], c_w2gt[:, :], elnst[:D, :], start=True, stop=True
                )
                vstk = bp.tile([D + 1, N], F16, tag="vstk")
                nc.scalar.activation(vstk[:D, :], psvT[:, :], AF.Copy)
                nc.vector.memset(vstk[D : D + 1, :], 1.0)

                Ef = bp.tile([N, D], F16, tag="Ef")
                nc.vector.tensor_scalar(Ef[:, :], E[:, :], 0.0, None, OP.add)

                # ---- block loop over j (software-pipelined: wm/reduce of
                # block t-1 are emitted during block t so Vector overlaps
                # Scalar's tanh and PE's next-block matmuls) ----
                z = bp.tile([N, N], F32, tag="z")
                atts = [None] * NBLK

                def emit_tail(t):
                    j0 = t * JB
                    jb = min(JB, N - j0)
                    F = jb * D
                    att = atts[t]
                    weng = nc.vector if WM_ENG[t] == "v" else nc.gpsimd
                    wm = blkp.tile([N, JB * D], F16, tag="wm")
                    weng.tensor_tensor(
                        wm[:, :F].rearrange("p (j m) -> p j m", m=D),
                        att[:, :F].rearrange("p (j m) -> p j m", m=D),
                        c_wrep[:, :].unsqueeze(1).broadcast_to([N, jb, D]),
                        OP.mult,
                    )
                    nc.vector.tensor_reduce(
                        z[:, j0 : j0 + jb],
                        wm[:, :F].rearrange("p (j m) -> p j m", m=D),
                        AX.X,
                        OP.add,
                    )

                def emit_head(t):
                    # mg build + MM1 for block t; returns psA tile
                    j0 = t * JB
                    jb = min(JB, N - j0)
                    F = jb * D
                    mg = blkp.tile([D, JB * D], F16, tag="mg")
                    eng = nc.gpsimd if MG_ENG[t] == "g" else nc.vector
                    eng.tensor_tensor(
                        mg[:, :F].rearrange("p (j m) -> p j m", m=D),
                        c_w3t[:, :].unsqueeze(1).broadcast_to([D, jb, D]),
                        Et[:, j0 : j0 + jb].unsqueeze(2).broadcast_to([D, jb, D]),
                        OP.mult,
                    )
                    psA = psAp.tile([N, JB * D], F32, tag="psA")
                    for h0 in range(0, F, 512):
                        h1 = min(h0 + 512, F)
                        nc.tensor.matmul(
                            psA[:, h0:h1], Et[:, :], mg[:, h0:h1],
                            start=True, stop=True,
                        )
                    return psA

                psAs = [None] * NBLK
                psAs[0] = emit_head(0)
                for t in range(NBLK):
                    j0 = t * JB
                    jb = min(JB, N - j0)
                    F = jb * D
                    psA = psAs[t]

                    X = blkp.tile([N, JB * D], F16, tag="X")
                    nc.vector.tensor_tensor(
                        X[:, :F].rearrange("p (j m) -> p j m", m=D),
                        psA[:, :F].rearrange("p (j m) -> p j m", m=D),
                        rGm1[:, j0 : j0 + jb].unsqueeze(2).broadcast_to([N, jb, D]),
                        OP.mult,
                    )
                    # next block's mg+MM1 go ahead of this block's uv/eyeX in
                    # the PE queue so the PE never stalls on X(t)
                    if t + 1 < NBLK:
                        psAs[t + 1] = emit_head(t + 1)
                    for h0 in range(0, F, 512):
                        h1 = min(h0 + 512, F)
                        nc.tensor.matmul(
                            psA[:, h0:h1], vstk[:, :],
                            mov[:, j0 * D + h0 : j0 * D + h1],
                            start=False, stop=False, skip_group_check=True,
                        )
                        nc.tensor.matmul(
                            psA[:, h0:h1], c_eye[:, :], X[:, h0:h1],
                            start=False, stop=True, skip_group_check=True,
                        )
                    att = blkp.tile([N, JB * D], F16, tag="att")
                    nc.scalar.activation(att[:, :F], psA[:, :F], AF.Tanh)
                    atts[t] = att
                    if t >= 1:
                        emit_tail(t - 1)
                emit_tail(NBLK - 1)

                # ---- softmax over j (free dim) ----
                zm = bp.tile([N, N], F32, tag="zm")
                nc.vector.tensor_tensor(zm[:, :], z[:, :], c_mask[:, :], OP.add)
                mx = bp.tile([N, 1], F32, tag="mx")
                nc.vector.tensor_reduce(mx[:, :], zm[:, :], AX.X, OP.max)
                mxn = bp.tile([N, 1], F32, tag="mxn")
                nc.vector.tensor_scalar_mul(mxn[:, :], mx[:, :], -1.0)
                p = bp.tile([N, N], F16, tag="p")
                nc.scalar.activation(p[:, :], zm[:, :], AF.Exp, bias=mxn[:, :])
                ssum = bp.tile([N, 1], F32, tag="ssum")
                nc.vector.tensor_reduce(ssum[:, :], p[:, :], AX.X, OP.add)
                sinv = bp.tile([N, 1], F32, tag="sinv")
                nc.vector.reciprocal(sinv[:, :], ssum[:, :])
                alph = bp.tile([N, N], F16, tag="alph")
                nc.vector.tensor_scalar_mul(alph[:, :], p[:, :], sinv[:, :])

                # ---- ctx = lrelu(E*(alphas@E + E)) ----
                psaT = psTp.tile([N, N], F16, tag="psT")
                nc.tensor.transpose(psaT[:, :], alph[:, :], c_eye[:, :])
                aT = bp.tile([N, N], F16, tag="aT")
                nc.scalar.activation(aT[:, :], psaT[:, :], AF.Copy)
                psc = psTp.tile([N, D], F32, tag="psu")
                nc.tensor.matmul(psc[:, :], aT[:, :], Ef[:, :], start=True, stop=True)
                s = bp.tile([N, D], F32, tag="s")
                nc.vector.tensor_tensor(s[:, :], psc[:, :], E[:, :], OP.add)
                ctxs = bp.tile([N, D], F32, tag="ctxs")
                nc.vector.tensor_tensor(ctxs[:, :], s[:, :], E[:, :], OP.mult)
                lo = bp.tile([N, D], F32, tag="lo")
                nc.vector.tensor_scalar_min(lo[:, :], ctxs[:, :], 0.0)
                hi = bp.tile([N, D], F32, tag="hi")
                nc.vector.tensor_scalar_max(hi[:, :], ctxs[:, :], 0.0)
                fin = bp.tile([N, D], F32, tag="fin")
                nc.vector.scalar_tensor_tensor(
                    fin[:, :], lo[:, :], NEG_SLOPE, hi[:, :], OP.mult, OP.add
                )
                nc.sync.dma_start(out=out[b, :, :], in_=fin[:, :])

    return nc


def _host_consts(ln_g, ln_b, w_ij, b_ij, w_sum, b_sum):
    d = D
    W1 = w_ij[:, :d]
    W2 = w_ij[:, d : 2 * d]
    W3 = w_ij[:, 2 * d :]
    W1g = W1 * ln_g[None, :]
    W2g = W2 * ln_g[None, :]
    W3g = W3 * ln_g[None, :]
    W3c = W3g - W3g.sum(axis=1)[:, None] / d
    Ctot = (W1 + W2 + W3) @ ln_b + b_ij
    w3t = np.ascontiguousarray(W3c.T)
    w1cp = np.vstack([W1g.T, Ctot[None, :]])
    w2gt = np.ascontiguousarray(W2g.T)
    indm = np.tile(np.eye(D, dtype=np.float16), (1, N))
    wrep = np.broadcast_to(w_sum[0][None, :], (N, D)).copy()
    maskd = (np.eye(N, dtype=np.float32) * -1e30).astype(np.float32)
    eye100 = np.eye(N, dtype=np.float16)
    return {
        "w3t": w3t.astype(np.float16),
        "w1cp": w1cp.astype(np.float16),
        "w2gt": w2gt.astype(np.float16),
        "indm": indm.astype(np.float16),
        "eye100": eye100,
        "wrep": wrep.astype(np.float16),
        "maskd": maskd,
    }


def _prepare(embeddings, ln_g, ln_b, w_ij, b_ij, w_sum, b_sum):
    embeddings = np.asarray(embeddings, dtype=np.float32)
    consts = _host_consts(
        np.asarray(ln_g, np.float32),
        np.asarray(ln_b, np.float32),
        np.asarray(w_ij, np.float32),
        np.asarray(b_ij, np.float32),
        np.asarray(w_sum, np.float32),
        np.asarray(b_sum, np.float32),
    )
    embt = np.ascontiguousarray(
        embeddings.transpose(0, 2, 1).astype(np.float16)
    )  # [BS, D, N]
    in_maps = []
    for c in range(NCORES):
        m = {
            "emb": np.ascontiguousarray(embeddings[c * BPC : (c + 1) * BPC]),
            "embt": np.ascontiguousarray(embt[c * BPC : (c + 1) * BPC]),
        }
        m.update(consts)
        in_maps.append(m)
    return in_maps


def kernel(embeddings, ln_g, ln_b, w_ij, b_ij, w_sum, b_sum, **kw):
    in_maps = _prepare(embeddings, ln_g, ln_b, w_ij, b_ij, w_sum, b_sum)
    if "nc" not in _CACHE:
        nc = build_program()
        if not nc.is_finalized():
            nc.finalize()
        _CACHE["nc"] = nc
    nc = _CACHE["nc"]
    res = run_bass_kernel_spmd(nc, in_maps, list(range(NCORES)))
    outs = [res.results[c]["out"] for c in range(NCORES)]
    return np.concatenate(outs, axis=0).astype(np.float32)


# revision 36
# speedup vs baseline: 1.0718x; 1.0529x over previous
"""Trainium2 Bass kernel for nn_ATT2 (gnn_message_passing).

Contract: kernel(**inputs) takes FULL unsharded inputs, returns FULL output.
Batch dim (64) is sharded 8-ways across cores; all params replicated.

Math (per batch, E [n=100, d=64]):
  eln      = LayerNorm_raw(E)        (affine folded into host consts)
  u'[j,m]  = sum_k W1g[m,k] eln[j,k] + Ctot[m]
  v'[i,m]  = sum_k W2g[m,k] eln[i,k]
  t3c[i,j,m] = sum_k W3c[m,k] E[i,k] E[j,k]   (W3c row-centered: LN mean
                                               term of pair-product cancels)
  r[i,j]   = rsqrt(var(e_i*e_j)+eps) via Gram matrices of E, E^2
  pre      = r*t3c + u'_j + v'_i
  z[i,j]   = sum_m w_sum[m] tanh(pre)    (b_sum softmax-invariant, dropped)
  alphas   = softmax_j(z, diag masked)
  ctx      = leaky_relu(e_i * (alphas @ E) + e_i^2)
           = leaky_relu(e_i * ((alphas @ E) + e_i))

Layout: i in partitions (100), free = (j, m) blocked over j (JB=16).
PSUM accumulation per block into one region psA:
  MM1:  psA  = t3c                  (lhsT=Et, rhs=mg[k,(j,m)]=W3cT[k,m]*Et[k,j])
  DVE:  X    = psA * (r-1)          (fp16 to SBUF; broadcast AP on rG-1)
  MM2:  psA += u'_j + v'_i          (lhsT=[v'T;ones], rhs=[ind_m;u'flat])
  MM3:  psA += eye100 @ X           (adds (r-1)*t3c => psA = r*t3c + u + v)
  ACT:  att  = tanh(psA)  (fp16)
  DVE:  wm   = att * w_bcast (fp16 2x); z[:,blk] = reduce_m(wm)
Softmax over j is free-dim (no transpose); ctx via alphas^T transpose matmul.
"""

import sys

sys.path.insert(0, "/opt/trn_rl_repo")

import numpy as np

import concourse.bass as bass
import concourse.bacc as bacc
import concourse.mybir as mybir
from concourse import tile
from concourse.bass_utils import run_bass_kernel_spmd

import ml_dtypes

F32 = mybir.dt.float32
F16 = mybir.dt.bfloat16
NPH = ml_dtypes.bfloat16
AF = mybir.ActivationFunctionType
OP = mybir.AluOpType
AX = mybir.AxisListType

D = 64
N = 100
BS = 64
NCORES = 8
BPC = BS // NCORES
LN_EPS = 1e-5
NEG_SLOPE = 0.01
JB = 16
NBLK = (N + JB - 1) // JB  # 7 (6 full + 1 of 4)

# engine per mg-build block: 'g' = gpsimd, 'v' = vector
MG_ENG = "g" * NBLK
# engine per wm block: 'v' = vector, 'g' = gpsimd (balance the two)
WM_ENG = "vgvgvgv"

_CACHE = {}

I32 = mybir.dt.int32
RSQRT_MAGIC = 0x5F3759DF


def _rsqrt(nc, pool, out, x, P, Fd, tag):
    """out = 1/sqrt(x) on the vector engine only (no ScalarE table set).

    Quake seed y0 = bitcast(magic - (bits(x) >> 1)), then two Newton
    passes y <- y * (1.5 - 0.5 * x * y^2)  (~1e-6 rel err for x in
    [1e-3, 1e3], plenty under the 2e-2 gate).
    """
    sh = pool.tile([P, Fd], I32, tag=f"{tag}_sh")
    nc.vector.tensor_scalar(
        sh[:, :], x[:, :].bitcast(I32), 1, None, OP.arith_shift_right
    )
    neg = pool.tile([P, Fd], I32, tag=f"{tag}_ng")
    nc.vector.tensor_scalar(
        neg[:, :], sh[:, :], -1, RSQRT_MAGIC, OP.mult, OP.add
    )
    y = neg[:, :].bitcast(F32)
    cur = y
    for it in range(2):
        y2 = pool.tile([P, Fd], F32, tag=f"{tag}_y2{it}")
        nc.vector.tensor_tensor(y2[:, :], cur, cur, OP.mult)
        xy2 = pool.tile([P, Fd], F32, tag=f"{tag}_xy{it}")
        nc.vector.tensor_tensor(xy2[:, :], x[:, :], y2[:, :], OP.mult)
        half = pool.tile([P, Fd], F32, tag=f"{tag}_hf{it}")
        nc.vector.tensor_scalar(
            half[:, :], xy2[:, :], -0.5, 1.5, OP.mult, OP.add
        )
        nxt = out if it == 1 else pool.tile([P, Fd], F32, tag=f"{tag}_nx{it}")
        nc.vector.tensor_tensor(
            nxt[:, :] if it == 1 else nxt[:, :], cur, half[:, :], OP.mult
        )
        cur = nxt[:, :]


def build_program():
    nc = bacc.Bacc()

    emb = nc.declare_dram_parameter("emb", [BPC, N, D], F32, isOutput=False)
    embt = nc.declare_dram_parameter("embt", [BPC, D, N], F16, isOutput=False)
    w3t = nc.declare_dram_parameter("w3t", [D, D], F16, isOutput=False)
    w1cp = nc.declare_dram_parameter("w1cp", [D + 1, D], F16, isOutput=False)
    w2gt = nc.declare_dram_parameter("w2gt", [D, D], F16, isOutput=False)
    indm = nc.declare_dram_parameter("indm", [D, N * D], F16, isOutput=False)
    eye100 = nc.declare_dram_parameter("eye100", [N, N], F16, isOutput=False)
    wrep = nc.declare_dram_parameter("wrep", [N, D], F16, isOutput=False)
    maskd = nc.declare_dram_parameter("maskd", [N, N], F32, isOutput=False)
    out = nc.declare_dram_parameter("out", [BPC, N, D], F32, isOutput=True)

    with tile.TileContext(nc) as tc:
        with (
            tc.tile_pool(name="const", bufs=1) as constp,
            tc.tile_pool(name="mov", bufs=1) as movp,
            tc.tile_pool(name="batch", bufs=2) as bp,
            tc.tile_pool(name="blk", bufs=3) as blkp,
            tc.tile_pool(name="ps_big", bufs=2, space="PSUM") as psAp,
            tc.tile_pool(name="ps_gram", bufs=1, space="PSUM") as psGp,
            tc.tile_pool(name="ps_sm", bufs=1, space="PSUM") as psTp,
            tc.tile_pool(name="dram", bufs=2, space="DRAM") as dramp,
        ):
            # ---- one-time constants ----
            c_w3t = constp.tile([D, D], F16, tag="c_w3t")
            nc.sync.dma_start(out=c_w3t[:, :], in_=w3t[:, :])
            c_w1cp = constp.tile([D + 1, D], F16, tag="c_w1cp")
            nc.sync.dma_start(out=c_w1cp[:, :], in_=w1cp[:, :])
            c_w2gt = constp.tile([D, D], F16, tag="c_w2gt")
            nc.sync.dma_start(out=c_w2gt[:, :], in_=w2gt[:, :])
            c_eye = constp.tile([N, N], F16, tag="c_eye")
            nc.sync.dma_start(out=c_eye[:, :], in_=eye100[:, :])
            c_wrep = constp.tile([N, D], F16, tag="c_wrep")
            nc.sync.dma_start(out=c_wrep[:, :], in_=wrep[:, :])
            c_mask = constp.tile([N, N], F32, tag="c_mask")
            nc.sync.dma_start(out=c_mask[:, :], in_=maskd[:, :])
            c_eps1 = constp.tile([N, 1], F32, tag="c_eps1")
            nc.vector.memset(c_eps1[:, :], LN_EPS)
            # movA/movB: rows 0..63 = indm const, row 64 = u'flat per batch
            movs = []
            for mi in range(2):
                mv = movp.tile([D + 1, N * D], F16, tag=f"mov{mi}")
                nc.sync.dma_start(out=mv[:D, :], in_=indm[:, :])
                movs.append(mv)

            for b in range(BPC):
                mov = movs[b % 2]
                # ---- loads ----
                E = bp.tile([N, D], F32, tag="E")
                nc.sync.dma_start(out=E[:, :], in_=emb[b, :, :])
                Et = bp.tile([D, N], F16, tag="Et")
                nc.sync.dma_start(out=Et[:, :], in_=embt[b, :, :])

                # ---- row stats & eln ----
                E2s = bp.tile([N, D], F32, tag="E2s")
                nc.vector.tensor_tensor(E2s[:, :], E[:, :], E[:, :], OP.mult)
                qsum = bp.tile([N, 1], F32, tag="qsum")
                nc.vector.tensor_reduce(qsum[:, :], E2s[:, :], AX.X, OP.add)
                msum = bp.tile([N, 1], F32, tag="msum")
                nc.vector.tensor_reduce(msum[:, :], E[:, :], AX.X, OP.add)
                mean = bp.tile([N, 1], F32, tag="mean")
                nc.vector.tensor_scalar_mul(mean[:, :], msum[:, :], 1.0 / D)
                msq = bp.tile([N, 1], F32, tag="msq")
                nc.vector.tensor_tensor(msq[:, :], mean[:, :], mean[:, :], OP.mult)
                var = bp.tile([N, 1], F32, tag="var")
                nc.vector.scalar_tensor_tensor(
                    var[:, :], qsum[:, :], 1.0 / D, msq[:, :], OP.mult, OP.subtract
                )
                vare = bp.tile([N, 1], F32, tag="vare")
                nc.vector.tensor_scalar(
                    vare[:, :], var[:, :], float(LN_EPS), None, OP.add
                )
                rstd = bp.tile([N, 1], F32, tag="rstd")
                _rsqrt(nc, bp, rstd, vare, N, 1, "rsA")
                eln = bp.tile([N, D], F16, tag="eln")
                nc.vector.tensor_scalar(
                    eln[:, :], E[:, :], mean[:, :], rstd[:, :], OP.subtract, OP.mult
                )

                # ---- transpose eln (PE), build elnst [65,100] ----
                psT1 = psTp.tile([D, N], F16, tag="psT")
                nc.tensor.transpose(psT1[:, :], eln[:, :], c_eye[:, :])
                elnst = bp.tile([D + 1, N], F16, tag="elnst")
                nc.scalar.activation(elnst[:D, :], psT1[:, :], AF.Copy)
                nc.vector.memset(elnst[D : D + 1, :], 1.0)
                E2t = bp.tile([D, N], F16, tag="E2t")
                nc.vector.tensor_tensor(E2t[:, :], Et[:, :], Et[:, :], OP.mult)

                # ---- pair-stats r via Gram matmuls ----
                psG = psGp.tile([N, 2 * N], F32, tag="psG")
                nc.tensor.matmul(psG[:, :N], Et[:, :], Et[:, :], start=True, stop=True)
                nc.tensor.matmul(
                    psG[:, N : 2 * N], E2t[:, :], E2t[:, :], start=True, stop=True
                )
                muG = bp.tile([N, N], F32, tag="muG")
                nc.vector.tensor_scalar_mul(muG[:, :], psG[:, :N], 1.0 / D)
                musq = bp.tile([N, N], F32, tag="musq")
                nc.vector.tensor_tensor(musq[:, :], muG[:, :], muG[:, :], OP.mult)
                varG = bp.tile([N, N], F32, tag="varG")
                nc.vector.scalar_tensor_tensor(
                    varG[:, :], psG[:, N : 2 * N], 1.0 / D, musq[:, :],
                    OP.mult, OP.subtract,
                )
                varGe = bp.tile([N, N], F32, tag="varGe")
                nc.vector.tensor_scalar(
                    varGe[:, :], varG[:, :], float(LN_EPS), None, OP.add
                )
                rGf = bp.tile([N, N], F32, tag="rGf")
                _rsqrt(nc, bp, rGf, varGe, N, N, "rsB")
                # r-1: psA keeps t3c; adding X=(r-1)*t3c + uv yields r*t3c+uv
                rGm1 = bp.tile([N, N], F16, tag="rGm1")
                nc.vector.tensor_scalar(
                    rGm1[:, :], rGf[:, :], 1.0, None, OP.subtract
                )

                # ---- u' (to mov row 64) and v'^T (to vstk) ----
                psu = psTp.tile([N, D], F32, tag="psu")
                nc.tensor.matmul(
                    psu[:, :], elnst[:, :], c_w1cp[:, :], start=True, stop=True
                )
                us = bp.tile([N, D], F16, tag="us")
                nc.scalar.activation(us[:, :], psu[:, :], AF.Copy)
                vd = dramp.tile([1, N * D], F16, tag="vd")
                nc.sync.dma_start(
                    out=vd[:, :].rearrange("p (j m) -> (p j) m", m=D),
                    in_=us[:, :],
                )
                nc.sync.dma_start(out=mov[D : D + 1, :], in_=vd[:, :])
                psvT = psTp.tile([D, N], F32, tag="psvT")
                nc.tensor.matmul(
                    psvT[:, :], c_w2gt[:, :], elnst[:D, :], start=True, stop=True
                )
                vstk = bp.tile([D + 1, N], F16, tag="vstk")
                nc.scalar.activation(vstk[:D, :], psvT[:, :], AF.Copy)
                nc.vector.memset(vstk[D : D + 1, :], 1.0)

                Ef = bp.tile([N, D], F16, tag="Ef")
                nc.vector.tensor_scalar(Ef[:, :], E[:, :], 0.0, None, OP.add)

                # ---- block loop over j (software-pipelined: wm/reduce of
                # block t-1 are emitted during block t so Vector overlaps
                # Scalar's tanh and PE's next-block matmuls) ----
                z = bp.tile([N, N], F32, tag="z")
                atts = [None] * NBLK

                def emit_tail(t):
                    j0 = t * JB
                    jb = min(JB, N - j0)
                    F = jb * D
                    att = atts[t]
                    weng = nc.vector if WM_ENG[t] == "v" else nc.gpsimd
                    wm = blkp.tile([N, JB * D], F16, tag="wm")
                    weng.tensor_tensor(
                        wm[:, :F].rearrange("p (j m) -> p j m", m=D),
                        att[:, :F].rearrange("p (j m) -> p j m", m=D),
                        c_wrep[:, :].unsqueeze(1).broadcast_to([N, jb, D]),
                        OP.mult,
                    )
                    nc.vector.tensor_reduce(
                        z[:, j0 : j0 + jb],
                        wm[:, :F].rearrange("p (j m) -> p j m", m=D),
                        AX.X,
                        OP.add,
                    )

                def emit_head(t):
                    # mg build + MM1 for block t; returns psA tile
                    j0 = t * JB
                    jb = min(JB, N - j0)
                    F = jb * D
                    mg = blkp.tile([D, JB * D], F16, tag="mg")
                    eng = nc.gpsimd if MG_ENG[t] == "g" else nc.vector
                    eng.tensor_tensor(
                        mg[:, :F].rearrange("p (j m) -> p j m", m=D),
                        c_w3t[:, :].unsqueeze(1).broadcast_to([D, jb, D]),
                        Et[:, j0 : j0 + jb].unsqueeze(2).broadcast_to([D, jb, D]),
                        OP.mult,
                    )
                    psA = psAp.tile([N, JB * D], F32, tag="psA")
                    for h0 in range(0, F, 512):
                        h1 = min(h0 + 512, F)
                        nc.tensor.matmul(
                            psA[:, h0:h1], Et[:, :], mg[:, h0:h1],
                            start=True, stop=True,
                        )
                    return psA

                psAs = [None] * NBLK
                psAs[0] = emit_head(0)
                for t in range(NBLK):
                    j0 = t * JB
                    jb = min(JB, N - j0)
                    F = jb * D
                    psA = psAs[t]

                    X = blkp.tile([N, JB * D], F16, tag="X")
                    nc.vector.tensor_tensor(
                        X[:, :F].rearrange("p (j m) -> p j m", m=D),
                        psA[:, :F].rearrange("p (j m) -> p j m", m=D),
                        rGm1[:, j0 : j0 + jb].unsqueeze(2).broadcast_to([N, jb, D]),
                        OP.mult,
                    )
                    # next block's mg+MM1 go ahead of this block's uv/eyeX in
                    # the PE queue so the PE never stalls on X(t)
                    if t + 1 < NBLK:
                        psAs[t + 1] = emit_head(t + 1)
                    for h0 in range(0, F, 512):
                        h1 = min(h0 + 512, F)
                        nc.tensor.matmul(
                            psA[:, h0:h1], vstk[:, :],
                            mov[:, j0 * D + h0 : j0 * D + h1],
                            start=False, stop=False, skip_group_check=True,
                        )
                        nc.tensor.matmul(
                            psA[:, h0:h1], c_eye[:, :], X[:, h0:h1],
                            start=False, stop=True, skip_group_check=True,
                        )
                    att = blkp.tile([N, JB * D], F16, tag="att")
                    nc.scalar.activation(att[:, :F], psA[:, :F], AF.Tanh)
                    atts[t] = att
                    if t >= 1:
                        emit_tail(t - 1)
                emit_tail(NBLK - 1)

                # ---- softmax over j (free dim) ----
                zm = bp.tile([N, N], F32, tag="zm")
                nc.vector.tensor_tensor(zm[:, :], z[:, :], c_mask[:, :], OP.add)
                mx = bp.tile([N, 1], F32, tag="mx")
                nc.vector.tensor_reduce(mx[:, :], zm[:, :], AX.X, OP.max)
                mxn = bp.tile([N, 1], F32, tag="mxn")
                nc.vector.tensor_scalar_mul(mxn[:, :], mx[:, :], -1.0)
                p = bp.tile([N, N], F16, tag="p")
                nc.scalar.activation(p[:, :], zm[:, :], AF.Exp, bias=mxn[:, :])
                ssum = bp.tile([N, 1], F32, tag="ssum")
                nc.vector.tensor_reduce(ssum[:, :], p[:, :], AX.X, OP.add)
                sinv = bp.tile([N, 1], F32, tag="sinv")
                nc.vector.reciprocal(sinv[:, :], ssum[:, :])
                alph = bp.tile([N, N], F16, tag="alph")
                nc.vector.tensor_scalar_mul(alph[:, :], p[:, :], sinv[:, :])

                # ---- ctx = lrelu(E*(alphas@E + E)) ----
                psaT = psTp.tile([N, N], F16, tag="psT")
                nc.tensor.transpose(psaT[:, :], alph[:, :], c_eye[:, :])
                aT = bp.tile([N, N], F16, tag="aT")
                nc.scalar.activation(aT[:, :], psaT[:, :], AF.Copy)
                psc = psTp.tile([N, D], F32, tag="psu")
                nc.tensor.matmul(psc[:, :], aT[:, :], Ef[:, :], start=True, stop=True)
                s = bp.tile([N, D], F32, tag="s")
                nc.vector.tensor_tensor(s[:, :], psc[:, :], E[:, :], OP.add)
                ctxs = bp.tile([N, D], F32, tag="ctxs")
                nc.vector.tensor_tensor(ctxs[:, :], s[:, :], E[:, :], OP.mult)
                lo = bp.tile([N, D], F32, tag="lo")
                nc.vector.tensor_scalar_min(lo[:, :], ctxs[:, :], 0.0)
                hi = bp.tile([N, D], F32, tag="hi")
                nc.vector.tensor_scalar_max(hi[:, :], ctxs[:, :], 0.0)
                fin = bp.tile([N, D], F32, tag="fin")
                nc.vector.scalar_tensor_tensor(
                    fin[:, :], lo[:, :], NEG_SLOPE, hi[:, :], OP.mult, OP.add
                )
                nc.sync.dma_start(out=out[b, :, :], in_=fin[:, :])

    return nc


def _host_consts(ln_g, ln_b, w_ij, b_ij, w_sum, b_sum):
    d = D
    W1 = w_ij[:, :d]
    W2 = w_ij[:, d : 2 * d]
    W3 = w_ij[:, 2 * d :]
    W1g = W1 * ln_g[None, :]
    W2g = W2 * ln_g[None, :]
    W3g = W3 * ln_g[None, :]
    W3c = W3g - W3g.sum(axis=1)[:, None] / d
    Ctot = (W1 + W2 + W3) @ ln_b + b_ij
    w3t = np.ascontiguousarray(W3c.T)
    w1cp = np.vstack([W1g.T, Ctot[None, :]])
    w2gt = np.ascontiguousarray(W2g.T)
    indm = np.tile(np.eye(D, dtype=np.float16), (1, N))
    wrep = np.broadcast_to(w_sum[0][None, :], (N, D)).copy()
    maskd = (np.eye(N, dtype=np.float32) * -1e30).astype(np.float32)
    eye100 = np.eye(N, dtype=np.float16)
    return {
        "w3t": w3t.astype(np.float16),
        "w1cp": w1cp.astype(np.float16),
        "w2gt": w2gt.astype(np.float16),
        "indm": indm.astype(np.float16),
        "eye100": eye100,
        "wrep": wrep.astype(np.float16),
        "maskd": maskd,
    }


def _prepare(embeddings, ln_g, ln_b, w_ij, b_ij, w_sum, b_sum):
    embeddings = np.asarray(embeddings, dtype=np.float32)
    consts = _host_consts(
        np.asarray(ln_g, np.float32),
        np.asarray(ln_b, np.float32),
        np.asarray(w_ij, np.float32),
        np.asarray(b_ij, np.float32),
        np.asarray(w_sum, np.float32),
        np.asarray(b_sum, np.float32),
    )
    embt = np.ascontiguousarray(
        embeddings.transpose(0, 2, 1).astype(np.float16)
    )  # [BS, D, N]
    in_maps = []
    for c in range(NCORES):
        m = {
            "emb": np.ascontiguousarray(embeddings[c * BPC : (c + 1) * BPC]),
            "embt": np.ascontiguousarray(embt[c * BPC : (c + 1) * BPC]),
        }
        m.update(consts)
        in_maps.append(m)
    return in_maps


def kernel(embeddings, ln_g, ln_b, w_ij, b_ij, w_sum, b_sum, **kw):
    in_maps = _prepare(embeddings, ln_g, ln_b, w_ij, b_ij, w_sum, b_sum)
    if "nc" not in _CACHE:
        nc = build_program()
        if not nc.is_finalized():
            nc.finalize()
        _CACHE["nc"] = nc
    nc = _CACHE["nc"]
    res = run_bass_kernel_spmd(nc, in_maps, list(range(NCORES)))
    outs = [res.results[c]["out"] for c in range(NCORES)]
    return np.concatenate(outs, axis=0).astype(np.float32)
